# revision 11
# baseline (speedup 1.0000x reference)
"""Trainium2 Bass kernel: causal GQA self-attention with RoPE + QK RMS-norm.

Model (full): x[B=2,T=2048,C=2048] -> q/k/v proj -> RoPE -> RMSNorm(q,k) ->
causal GQA attention (16 q heads, 4 kv heads) -> out proj wproj.

Sharding over 8 NeuronCores: core = 4*b + g, b in {0,1} batch, g in {0..3}
kv-group. Each core handles one batch's kv head g and its 4 q heads
(h = 4g..4g+3), producing the partial c_proj output
y_heads @ wproj[:, 512g:512(g+1)].T of shape [T, C]. The host sums the 4
partials per batch (the "all-reduce after c_proj" done at gather time).

Device layouts (prepped on host):
  xT   [C, T]      x[b] transposed        (contraction dim c on partitions)
  wqT  [C, 512]    wq rows for 4 q heads, transposed
  wkvT [C, 256]    [wk_g ; wv_g] transposed
  wpT  [512, C]    wproj columns for the group, transposed
  cos/sin [T, 128] rope tables
  mask [128,128]   upper-tri (tk<=tq) 0/1 for diagonal blocks
  ident [128,128]  identity for PE transposes

In-kernel dataflow per core (all matmuls fp32r, PSUM f32 accumulate):
  stage A per t-tile: Q/K/V projections (lhsT = xT tile, moving = weights),
    RoPE + RMSNorm on natural [t,d] tiles, PE-transpose normalized Q/K to
    [d,t] layout.
  stage B per (tq-slice j, head h): S^T = K_tile^T . Q  -> exp on ACT ->
    mask diag -> denominators via ones-matmul -> AV accumulate (Y^T).
  stage C per j: c_proj with lhsT = normalized Y^T, natural [t, e] output.
Causality: tk-tile i contributes to tq-slice j only for i <= 4j+3; partial
blocks restrict to the valid column suffix.
"""

import math
from contextlib import ExitStack

import numpy as np

import concourse.bass as bass
import concourse.mybir as mybir
import concourse.tile as tile
from concourse import bacc
from concourse.bass import ts
from concourse.bass_utils import run_bass_kernel_spmd

F32 = mybir.dt.float32
N_HEAD = 16
N_KV = 4
D = 128
RMS_EPS = float(np.finfo(np.float32).eps)
SCALE = 1.0 / math.sqrt(D)


def build_bass(T=2048, C=2048, HQ=4, E=2048, rep=1, dt=mybir.dt.float32r, stages='ABC'):
    """One core's program. T,C,E multiples of 512; HQ q-heads (1 kv head)."""
    TT, CT, NE, TQ = T // 128, C // 128, E // 512, T // 512
    HD = HQ * 128

    nc = bacc.Bacc("TRN2", target_bir_lowering=False)
    xT_d = nc.dram_tensor("xT", [C, T], dt, kind="ExternalInput")
    wqT_d = nc.dram_tensor("wqT", [C, HD], dt, kind="ExternalInput")
    wkvT_d = nc.dram_tensor("wkvT", [C, 256], dt, kind="ExternalInput")
    wpT_d = nc.dram_tensor("wpT", [HD, E], dt, kind="ExternalInput")
    cos_d = nc.dram_tensor("cosd", [T, D], F32, kind="ExternalInput")
    sin_d = nc.dram_tensor("sind", [T, D], F32, kind="ExternalInput")
    mask_d = nc.dram_tensor("maskd", [128, 128], dt, kind="ExternalInput")
    id_d = nc.dram_tensor("identd", [128, 128], dt, kind="ExternalInput")
    out_d = nc.dram_tensor("out", [T, E], F32, kind="ExternalOutput")

    with tile.TileContext(nc) as tc, ExitStack() as ctx:
        P = lambda **kw: ctx.enter_context(tc.tile_pool(**kw))
        wp = P(name="w", bufs=1)            # persistent weights/constants
        xp = P(name="x", bufs=2)            # xT strips
        csp = P(name="cs", bufs=2)          # cos/sin tiles
        rp = P(name="rope", bufs=2)         # rope scratch
        qnp = P(name="qn", bufs=2)          # normalized q/k (pre-transpose)
        pp = P(name="p", bufs=3)            # exp(P) tiles
        bp = P(name="bc", bufs=2)           # denominators / bcast
        yp = P(name="y", bufs=1)            # per-j YT
        op = P(name="o", bufs=2)            # output staging
        ps_s = P(name="ps_s", bufs=3, space="PSUM")   # proj / scores / cproj
        ps_t = P(name="ps_t", bufs=2, space="PSUM")   # transposes
        ps_a = P(name="ps_a", bufs=2, space="PSUM")   # AV accumulators
        ps_d = P(name="ps_d", bufs=1, space="PSUM")   # denominators

        # persistent SBUF
        wq_s = wp.tile([128, CT, HD], dt)
        nc.sync.dma_start(wq_s, wqT_d.ap().rearrange("(n p) m -> p n m", p=128))
        wkv_s = wp.tile([128, CT, 256], dt)
        nc.sync.dma_start(wkv_s, wkvT_d.ap().rearrange("(n p) m -> p n m", p=128))
        wp_s = wp.tile([128, HQ, E], dt)
        nc.sync.dma_start(wp_s, wpT_d.ap().rearrange("(n p) m -> p n m", p=128))
        mask_s = wp.tile([128, 128], dt)
        nc.sync.dma_start(mask_s, mask_d.ap())
        ident = wp.tile([128, 128], dt)
        nc.sync.dma_start(ident, id_d.ap())
        ONE_F32_BITS = 0x3F800000
        ones_c = wp.tile([128, 1], dt)
        nc.vector.memset(ones_c.bitcast(mybir.dt.uint32), ONE_F32_BITS)
        ones_r = wp.tile([1, 128], dt)
        nc.vector.memset(ones_r.bitcast(mybir.dt.uint32), ONE_F32_BITS)
        eps_s = wp.tile([128, 1], F32)
        nc.vector.memset(eps_s, RMS_EPS)

        def bcast(ap, axis, n):
            """Insert a stride-0 dim of size n at free-axis position `axis`."""
            a = list(ap.ap)
            a.insert(axis, [0, n])
            return bass.AP(tensor=ap.tensor, offset=ap.offset, ap=a)

        for _ in range(rep):
            # persistent-per-rep activation tiles (distinct tags)
            qT = {}  # (h, j) -> [128, 4, 128] tile, d-major
            kT = []  # i -> [128, 128]
            vS = []  # i -> [128, 128]
            for h in range(HQ):
                for j in range(TQ):
                    qT[(h, j)] = wp.tile([128, 4, 128], dt, tag=f"qT{h}_{j}", name=f"qT{h}_{j}")
            for i in range(TT):
                kT.append(wp.tile([128, 128], dt, tag=f"kT{i}", name=f"kT{i}"))
                vS.append(wp.tile([128, 128], dt, tag=f"vS{i}", name=f"vS{i}"))

            # ---- stage A: projections + rope + rms + transpose ----
            xT_r = xT_d.ap().rearrange("(n p) t -> p n t", p=128)
            for i in range(TT):
                xs = xp.tile([128, CT, 128], dt)
                nc.sync.dma_start(xs, xT_r[:, :, ts(i, 128)])
                cst = csp.tile([128, D], F32, tag="cos")
                nc.sync.dma_start(cst, cos_d.ap()[ts(i, 128), :])
                snt = csp.tile([128, D], F32, tag="sin")
                nc.sync.dma_start(snt, sin_d.ap()[ts(i, 128), :])

                pq = ps_s.tile([128, HD], F32, tag="s")
                for c in range(CT):
                    nc.tensor.matmul(pq, xs[:, c], wq_s[:, c],
                                     start=(c == 0), stop=(c == CT - 1))
                pkv = ps_s.tile([128, 256], F32, tag="s")
                for c in range(CT):
                    nc.tensor.matmul(pkv, xs[:, c], wkv_s[:, c],
                                     start=(c == 0), stop=(c == CT - 1))
                nc.scalar.copy(vS[i], pkv[:, 128:256])

                def rope_rms(src, nh, dst_list):
                    """src: psum AP viewed [128, nh, 128]; writes dt tiles."""
                    ro = rp.tile([128, nh, D], F32, tag=f"ro{nh}")
                    nc.vector.tensor_mul(ro, src, bcast(cst[:, :], 1, nh))
                    tmp = rp.tile([128, nh, 64], F32, tag=f"tm{nh}")
                    nc.vector.tensor_mul(tmp, src[:, :, 64:128],
                                         bcast(snt[:, 0:64], 1, nh))
                    nc.vector.tensor_sub(ro[:, :, 0:64], ro[:, :, 0:64], tmp)
                    tmp2 = rp.tile([128, nh, 64], F32, tag=f"t2{nh}")
                    nc.vector.tensor_mul(tmp2, src[:, :, 0:64],
                                         bcast(snt[:, 64:128], 1, nh))
                    nc.vector.tensor_add(ro[:, :, 64:128], ro[:, :, 64:128], tmp2)
                    sq = rp.tile([128, nh], F32, tag=f"sq{nh}")
                    scr = rp.tile([128, nh, D], F32, tag=f"sc{nh}")
                    nc.vector.tensor_mul(scr, ro, ro)
                    nc.vector.reduce_sum(sq, scr, axis=mybir.AxisListType.X)
                    rs = rp.tile([128, nh], F32, tag=f"rs{nh}")
                    nc.scalar.activation(rs, sq,
                                         mybir.ActivationFunctionType.Sqrt,
                                         bias=eps_s[:, :], scale=1.0 / D)
                    rr = rp.tile([128, nh], F32, tag=f"rr{nh}")
                    nc.vector.reciprocal(rr, rs)
                    qn = qnp.tile([128, nh, D], dt, tag=f"qn{nh}")
                    for h in range(nh):
                        nc.vector.tensor_scalar_mul(qn[:, h], ro[:, h],
                                                    rr[:, h:h + 1])
                    for h in range(nh):
                        pt = ps_t.tile([128, 128], dt)
                        nc.tensor.transpose(pt, qn[:, h], ident)
                        nc.scalar.copy(dst_list[h], pt)

                j, tsub = i // 4, i % 4
                rope_rms(pq[:].rearrange("p (h d) -> p h d", d=D), HQ,
                         [qT[(h, j)][:, tsub] for h in range(HQ)])
                rope_rms(pkv[:, 0:128].rearrange("p (h d) -> p h d", d=D), 1,
                         [kT[i]])

            # ---- stage B + C per tq-slice ----
            if stages == 'A':
                dbg = op.tile([128, 512], F32, tag="ot")
                nc.vector.tensor_copy(dbg[:, 0:128], kT[0])
                nc.sync.dma_start(out_d.ap()[0:128, 0:512], dbg)
                continue
            for j in range(TQ):
                ynj = yp.tile([128, HQ, 4, 128], dt)
                for h in range(HQ):
                    nblk = 4 * j + 4
                    pav = ps_a.tile([128, 512], F32)
                    pd = ps_d.tile([1, 512], F32)
                    for i in range(nblk):
                        ai = max(0, i - 4 * j) * 128
                        psb = ps_s.tile([128, 512], F32, tag="s")
                        nc.tensor.matmul(psb[:, ai:512], kT[i],
                                         qT[(h, j)][:, ai // 128:4])
                        pe = pp.tile([128, 512], dt)
                        nc.scalar.activation(pe[:, ai:512], psb[:, ai:512],
                                             mybir.ActivationFunctionType.Exp,
                                             scale=SCALE)
                        if i >= 4 * j:
                            nc.vector.tensor_mul(pe[:, ai:ai + 128],
                                                 pe[:, ai:ai + 128], mask_s)
                        nc.tensor.matmul(pd[:, ai:512], ones_c, pe[:, ai:512],
                                         start=(i == 0), stop=(i == nblk - 1))
                        nc.tensor.matmul(pav[:, ai:512], vS[i], pe[:, ai:512],
                                         start=(i == 0), stop=(i == nblk - 1))
                    rd = bp.tile([1, 512], F32, tag="rd")
                    nc.vector.reciprocal(rd, pd)
                    rdr = bp.tile([1, 512], dt, tag="rdr")
                    nc.scalar.copy(rdr, rd)
                    pb = ps_s.tile([128, 512], F32, tag="s")
                    nc.tensor.matmul(pb, ones_r, rdr)
                    bc = bp.tile([128, 512], F32, tag="bc")
                    nc.scalar.copy(bc, pb)
                    nc.vector.tensor_mul(
                        ynj[:, h].rearrange("p a b -> p (a b)"), pav, bc)
                if stages == 'AB':
                    dbg2 = op.tile([128, 512], F32, tag="ot")
                    nc.vector.tensor_copy(dbg2, ynj[:, 0].rearrange("p a b -> p (a b)"))
                    nc.sync.dma_start(out_d.ap()[ts(j, 128), 0:512], dbg2)
                    continue
                for tsub in range(4):
                    for e in range(NE):
                        pc = ps_s.tile([128, 512], F32, tag="s")
                        for h in range(HQ):
                            nc.tensor.matmul(pc, ynj[:, h, tsub],
                                             wp_s[:, h, ts(e, 512)],
                                             start=(h == 0), stop=(h == HQ - 1))
                        ot = op.tile([128, 512], F32, tag="ot")
                        nc.scalar.copy(ot, pc)
                        nc.sync.dma_start(
                            out_d.ap()[512 * j + 128 * tsub:
                                       512 * j + 128 * tsub + 128,
                                       ts(e, 512)], ot)
    nc.compile()
    return nc


def make_core_inputs(x, cos, sin, wq, wk, wv, wproj):
    """Full inputs -> list of 8 per-core input dicts (host-side sharding)."""
    x = np.asarray(x, dtype=np.float32)
    cos2 = np.ascontiguousarray(np.asarray(cos, np.float32).reshape(-1, D))
    sin2 = np.ascontiguousarray(np.asarray(sin, np.float32).reshape(-1, D))
    wq = np.asarray(wq, np.float32)
    wk = np.asarray(wk, np.float32)
    wv = np.asarray(wv, np.float32)
    wproj = np.asarray(wproj, np.float32)
    B = x.shape[0]
    mask = np.triu(np.ones((128, 128), np.float32))  # mask[r,q]=1 iff r<=q
    ident = np.eye(128, dtype=np.float32)
    in_maps = []
    xTs = [np.ascontiguousarray(x[b].T) for b in range(B)]
    for b in range(B):
        for g in range(N_KV):
            wqT = np.ascontiguousarray(wq[512 * g:512 * g + 512].T)
            wkvT = np.ascontiguousarray(
                np.concatenate([wk[128 * g:128 * g + 128],
                                wv[128 * g:128 * g + 128]], axis=0).T)
            wpT = np.ascontiguousarray(wproj[:, 512 * g:512 * g + 512].T)
            in_maps.append({
                "xT": xTs[b], "wqT": wqT, "wkvT": wkvT, "wpT": wpT,
                "cosd": cos2, "sind": sin2, "maskd": mask, "identd": ident,
            })
    return in_maps


_NC_CACHE = {}


def kernel(x, cos, sin, wq, wk, wv, wproj):
    x = np.asarray(x, dtype=np.float32)
    B, T, C = x.shape
    key = (T, C)
    if key not in _NC_CACHE:
        _NC_CACHE[key] = build_bass(T=T, C=C)
    nc = _NC_CACHE[key]
    in_maps = make_core_inputs(x, cos, sin, wq, wk, wv, wproj)
    res = run_bass_kernel_spmd(nc, in_maps, core_ids=list(range(8)))
    out = np.zeros((B, T, C), dtype=np.float64)
    for b in range(B):
        for g in range(N_KV):
            out[b] += res.results[4 * b + g]["out"].astype(np.float64)
    return out.astype(np.float32)
